# revision 1
# baseline (speedup 1.0000x reference)
"""CopyGenerator kernel for 8 trn2 NeuronCores (vocab-tensor-parallel).

Math (per reference):
    cp      = sigmoid(hidden @ w_copy + b_copy)            # copy gate, per token
    logits  = hidden @ W_gen.T + b_gen                     # [tok, V]
    prob    = softmax(logits)
    attn    = softmax(mask(hidden @ context.T per batch))  # [tok, S]
    p_g     = prob*(1-cp); p_g[t,b,src[b,s]] += attn*cp
    out     = log(p_g) + C

Sharding: vocab split 8 ways (4000/core + 32 pad/dup columns). Cross-core
softmax denominator via a tiny AllReduce. The scatter-add is made SPMD-uniform
by a per-core host-side *permutation* of the vocab columns: the vocab value
src[b,s] owned by a core is placed at column b*64+s, so the scatter is the
same 32 dense [64,64] adds on every core; ownership is zeroed via a mask
input. Vocab values hit by several (b,s) pairs get duplicate columns (merged
on the host afterwards).

Token layout is batch-outer: n = b*64 + t.
"""

import sys
import time

sys.path.insert(0, "/opt/trn_rl_repo")

import numpy as np

import concourse.bass as bass
import concourse.mybir as mybir
import concourse.tile as tile
from bass_rust import SyncInfo
from concourse.bass_utils import run_bass_kernel_spmd

FP32 = mybir.dt.float32
BF16 = mybir.dt.bfloat16
AF = mybir.ActivationFunctionType
OP = mybir.AluOpType

NCORE = 8
T, B, S, H, V = 64, 32, 64, 1024, 32000
NTOK = T * B              # 2048
KT = H // 128             # 8 k-tiles
VS = V // NCORE           # 4000 vocab / core
WCOLS = 4032              # 4000 + 32 dup/pad columns
VN = WCOLS // 8           # 504 per v-tile
TT_N = NTOK // 128        # 16 token tiles
C_CONST = 0.1712209
NEG_BIG = -1.0e30


def _split_multi_waits(nc):
    """This container's walrus accepts at most 1 sem-wait per instruction
    (2 on EventSemaphore). Tile's exit drain exceeds that; hoist extras onto
    EventSemaphore carriers inserted right before the offender."""
    for f in nc.m.functions:
        for b in f.blocks:
            out, changed = [], False
            for inst in list(b.instructions):
                si = inst.sync_info
                if si is not None:
                    waits = list(si.on_wait)
                    cap = 2 if isinstance(inst, mybir.InstEventSemaphore) else 1
                    if len(waits) > cap:
                        extra = waits[: len(waits) - cap]
                        keep = waits[len(waits) - cap:]
                        for k in range(0, len(extra), 2):
                            es = mybir.InstEventSemaphore(
                                name=f"{inst.name}_xw{k}", ins=[], outs=[])
                            es.engine = inst.engine
                            es.sync_info = SyncInfo(
                                on_wait=extra[k:k + 2], on_update=[])
                            nc.register_instruction(es)
                            out.append(es)
                        inst.sync_info = SyncInfo(
                            on_wait=keep, on_update=list(si.on_update))
                        changed = True
                out.append(inst)
            if changed:
                b.instructions = out


def build_program():
    """One SPMD program; all data-dependence is in the input tensors."""
    nc = bass.Bass("TRN2", target_bir_lowering=False, debug=False,
                   num_devices=NCORE)

    hT = nc.dram_tensor("hT", [H, NTOK], FP32, kind="ExternalInput")
    cT = nc.dram_tensor("cT", [H, NTOK], FP32, kind="ExternalInput")
    wT = nc.dram_tensor("wT", [H, WCOLS], FP32, kind="ExternalInput")
    wcp = nc.dram_tensor("wcp", [128, KT], FP32, kind="ExternalInput")
    bcp = nc.dram_tensor("bcp", [128, 1], FP32, kind="ExternalInput")
    amask = nc.dram_tensor("amask", [1, NTOK], FP32, kind="ExternalInput")
    omask = nc.dram_tensor("omask", [128, TT_N * S], FP32, kind="ExternalInput")
    out = nc.dram_tensor("out", [NTOK, WCOLS], FP32, kind="ExternalOutput")

    z_in = nc.dram_tensor("z_in", [128, TT_N], FP32)
    z_out = nc.dram_tensor("z_out", [128, TT_N], FP32, addr_space="Shared")

    with tile.TileContext(nc) as tc:
        with tc.tile_pool(name="pers", bufs=1) as pers:
            # persistent smalls
            wcp_sb = pers.tile([128, KT], FP32, name="wcp_sb", tag="wcp_sb")
            nc.sync.dma_start(wcp_sb[:], wcp[:])
            bcp_sb = pers.tile([128, 1], FP32, name="bcp_sb", tag="bcp_sb")
            nc.sync.dma_start(bcp_sb[:], bcp[:])
            amask_sb = pers.tile([1, NTOK], FP32, name="amask_sb", tag="amask_sb")
            nc.sync.dma_start(amask_sb[:], amask[:])
            omask_sb = pers.tile([128, TT_N * S], FP32, name="omask_sb",
                                 tag="omask_sb")
            nc.sync.dma_start(omask_sb[:], omask[:])
            ones_sb = pers.tile([1, S], FP32, name="ones_sb", tag="ones_sb")
            nc.vector.memset(ones_sb[:], 1.0)

            g_all = pers.tile([128, TT_N], FP32, name="g_all", tag="g_all")
            omcp_all = pers.tile([128, TT_N], FP32, name="omcp_all",
                                 tag="omcp_all")
            l1m_all = pers.tile([128, TT_N], FP32, name="l1m_all", tag="l1m_all")
            sfin_all = pers.tile([128, TT_N], FP32, name="sfin_all",
                                 tag="sfin_all")
            zall = pers.tile([128, TT_N], FP32, name="zall", tag="zall")
            zz = pers.tile([128, TT_N], FP32, name="zz", tag="zz")

            hTb = [pers.tile([128, NTOK], BF16, name=f"hTb{k}", tag=f"hTb{k}")
                   for k in range(KT)]
            pc_t = [pers.tile([128, S], FP32, name=f"pc{t}", tag=f"pc{t}")
                    for t in range(TT_N)]
            zparts = [pers.tile([128, 8], FP32, name=f"zp{t}", tag=f"zp{t}")
                      for t in range(TT_N)]

            # ---------------- Phase A: copy gate + attention (fp32) --------
            with (
                tc.tile_pool(name="hf", bufs=1) as hf,
                tc.tile_pool(name="psA", bufs=2, space="PSUM") as psA,
                tc.tile_pool(name="attw", bufs=3) as attw,
            ):
                hTf = []
                cTf = []
                for k in range(KT):
                    hfk = hf.tile([128, NTOK], FP32, name=f"hTf{k}",
                                  tag=f"hTf{k}")
                    nc.sync.dma_start(hfk[:], hT[k * 128:(k + 1) * 128, :])
                    hTf.append(hfk)
                    nc.vector.tensor_copy(hTb[k][:], hfk[:])
                    cfk = hf.tile([128, NTOK], FP32, name=f"cTf{k}",
                                  tag=f"cTf{k}")
                    nc.sync.dma_start(cfk[:], cT[k * 128:(k + 1) * 128, :])
                    cTf.append(cfk)

                for tt in range(TT_N):
                    ns = slice(tt * 128, (tt + 1) * 128)
                    # copy-gate logit for this token tile: [128, 1]
                    pcl = psA.tile([128, 1], FP32, name="pcl", tag="pcl")
                    for k in range(KT):
                        nc.tensor.matmul(pcl[:], lhsT=hTf[k][:, ns],
                                         rhs=wcp_sb[:, k:k + 1],
                                         start=(k == 0), stop=(k == KT - 1))
                    nc.scalar.activation(g_all[:, tt:tt + 1], pcl[:], AF.Exp,
                                         bias=bcp_sb[:], scale=1.0)
                    cp_col = attw.tile([128, 1], FP32, name="cp_col",
                                       tag="cp_col")
                    nc.scalar.activation(cp_col[:], pcl[:], AF.Sigmoid,
                                         bias=bcp_sb[:], scale=1.0)
                    nc.vector.tensor_scalar(
                        out=omcp_all[:, tt:tt + 1], in0=cp_col[:],
                        scalar1=-1.0, scalar2=1.0, op0=OP.mult, op1=OP.add)
                    nc.scalar.activation(l1m_all[:, tt:tt + 1],
                                         omcp_all[:, tt:tt + 1], AF.Ln,
                                         bias=0.0, scale=1.0)

                    # attention scores for the 2 batches of this token tile
                    pat = psA.tile([128, S], FP32, name="pat", tag="pat")
                    for half in range(2):
                        b = 2 * tt + half
                        rs = slice(64 * half, 64 * half + 64)
                        cs = slice(b * 64, (b + 1) * 64)
                        for k in range(KT):
                            nc.tensor.matmul(pat[rs, :], lhsT=hTf[k][:, cs],
                                             rhs=cTf[k][:, cs],
                                             start=(k == 0), stop=False)
                        nc.tensor.matmul(pat[rs, :], lhsT=ones_sb[:],
                                         rhs=amask_sb[:, cs],
                                         start=False, stop=True)
                    negmax = attw.tile([128, 1], FP32, name="negmax",
                                       tag="negmax")
                    nc.vector.tensor_reduce(negmax[:], pat[:],
                                            axis=mybir.AxisListType.X,
                                            op=OP.max, negate=True)
                    att_e = attw.tile([128, S], FP32, name="att_e", tag="att_e")
                    rowsum = attw.tile([128, 1], FP32, name="rowsum",
                                       tag="rowsum")
                    nc.scalar.activation(att_e[:], pat[:], AF.Exp,
                                         bias=negmax[:], scale=1.0,
                                         accum_out=rowsum[:])
                    rec = attw.tile([128, 1], FP32, name="rec", tag="rec")
                    nc.vector.reciprocal(rec[:], rowsum[:])
                    pg = attw.tile([128, 1], FP32, name="pg", tag="pg")
                    nc.vector.tensor_tensor(out=pg[:], in0=rec[:],
                                            in1=g_all[:, tt:tt + 1],
                                            op=OP.mult)
                    # pc = attns * cp/(1-cp) * ownership-mask
                    nc.vector.tensor_scalar(out=pc_t[tt][:], in0=att_e[:],
                                            scalar1=pg[:], scalar2=None,
                                            op0=OP.mult)
                    nc.vector.tensor_tensor(
                        out=pc_t[tt][:], in0=pc_t[tt][:],
                        in1=omask_sb[:, tt * S:(tt + 1) * S], op=OP.mult)

            # ---------------- Phase B: big matmul + exp + z ----------------
            ebuf = [None] * TT_N
            with (
                tc.tile_pool(name="eb", bufs=1) as eb,
                tc.tile_pool(name="wstream", bufs=3) as wstream,
                tc.tile_pool(name="wtbp", bufs=12) as wtbp,
                tc.tile_pool(name="psB", bufs=5, space="PSUM") as psB,
                tc.tile_pool(name="outp", bufs=3) as outp,
                tc.tile_pool(name="post", bufs=2) as post,
            ):
                for tt in range(TT_N):
                    ebuf[tt] = eb.tile([128, WCOLS], BF16, name=f"E{tt}",
                                       tag=f"E{tt}")

                for vt in range(8):
                    vsl = slice(vt * VN, (vt + 1) * VN)
                    wb = []
                    for k in range(KT):
                        ws = wstream.tile([128, VN], FP32, name="ws", tag="ws")
                        nc.sync.dma_start(ws[:],
                                          wT[k * 128:(k + 1) * 128, vsl])
                        wbk = wtbp.tile([128, VN], BF16, name="wb", tag="wb")
                        nc.vector.tensor_copy(wbk[:], ws[:])
                        wb.append(wbk)
                    for tt in range(TT_N):
                        ns = slice(tt * 128, (tt + 1) * 128)
                        ps = psB.tile([128, VN], FP32, name="mmp", tag="mmp")
                        for k in range(KT):
                            nc.tensor.matmul(ps[:], lhsT=hTb[k][:, ns],
                                             rhs=wb[k][:],
                                             start=(k == 0), stop=(k == KT - 1))
                        nc.scalar.activation(
                            ebuf[tt][:, vsl], ps[:], AF.Exp,
                            bias=l1m_all[:, tt:tt + 1], scale=1.0,
                            accum_out=zparts[tt][:, vt:vt + 1])

                # local z, allreduce, final scale
                for tt in range(TT_N):
                    nc.vector.tensor_reduce(zall[:, tt:tt + 1], zparts[tt][:],
                                            axis=mybir.AxisListType.X,
                                            op=OP.add)
                nc.sync.dma_start(z_in[:], zall[:])
                nc.gpsimd.collective_compute(
                    "AllReduce", OP.add,
                    replica_groups=[list(range(NCORE))],
                    ins=[z_in[:]], outs=[z_out[:]])
                nc.sync.dma_start(zz[:], z_out[:])

                ecc = float(np.exp(C_CONST))
                for tt in range(TT_N):
                    zr = post.tile([128, 1], FP32, name="zr", tag="zr")
                    nc.vector.reciprocal(zr[:], zz[:, tt:tt + 1])
                    nc.vector.tensor_scalar(
                        out=sfin_all[:, tt:tt + 1], in0=zr[:],
                        scalar1=omcp_all[:, tt:tt + 1], scalar2=ecc,
                        op0=OP.mult, op1=OP.mult)
                    pcz = post.tile([128, S], BF16, name="pcz", tag="pcz")
                    nc.vector.tensor_scalar(
                        out=pcz[:], in0=pc_t[tt][:],
                        scalar1=zz[:, tt:tt + 1], scalar2=None, op0=OP.mult)
                    # the scatter: uniform dense adds into the landing zone
                    for half in range(2):
                        b = 2 * tt + half
                        rs = slice(64 * half, 64 * half + 64)
                        cs = slice(b * 64, (b + 1) * 64)
                        nc.vector.tensor_tensor(
                            out=ebuf[tt][rs, cs], in0=ebuf[tt][rs, cs],
                            in1=pcz[rs, :], op=OP.add)

                # final log + store
                for tt in range(TT_N):
                    ns = slice(tt * 128, (tt + 1) * 128)
                    for vt in range(8):
                        vsl = slice(vt * VN, (vt + 1) * VN)
                        ob = outp.tile([128, VN], FP32, name="ob", tag="ob")
                        nc.scalar.activation(ob[:], ebuf[tt][:, vsl], AF.Ln,
                                             bias=0.0,
                                             scale=sfin_all[:, tt:tt + 1])
                        nc.sync.dma_start(out[ns, vsl], ob[:])

    _split_multi_waits(nc)
    return nc


# ----------------------------------------------------------------------------
# host-side sharding / permutation / assembly
# ----------------------------------------------------------------------------

def _prep_inputs(hidden, context, src, W_gen, b_gen, w_copy, b_copy):
    assert hidden.shape == (T, B, H) and context.shape == (S, B, H)
    assert W_gen.shape == (V, H) and src.shape == (B, S)
    if not np.all(np.asarray(b_gen) == 0.0):
        raise NotImplementedError("b_gen expected to be all zeros per spec")

    hT = np.ascontiguousarray(
        hidden.astype(np.float32).transpose(2, 1, 0).reshape(H, NTOK))
    cT = np.ascontiguousarray(
        context.astype(np.float32).transpose(2, 1, 0).reshape(H, NTOK))
    wcp = np.ascontiguousarray(
        w_copy.astype(np.float32).reshape(KT, 128).T)
    bcp = np.full((128, 1), float(np.asarray(b_copy).reshape(-1)[0]),
                  np.float32)

    src = np.asarray(src).astype(np.int64)
    amask = np.where(src.reshape(1, NTOK) == 0, np.float32(NEG_BIG),
                     np.float32(0.0))  # [1, B*S] with col b*64+s

    per_core = []
    for c in range(NCORE):
        lo, hi = c * VS, (c + 1) * VS
        # column assignment
        col_vocab = np.full(WCOLS, -1, np.int64)   # vocab id per column
        placed = {}                                # vocab id -> primary col
        own_pairs = []                             # (b, s, col)
        dup_info = []                              # (b, col, vocab, primary)
        for b in range(B):
            for s in range(S):
                v = int(src[b, s])
                if v == 0 or not (lo <= v < hi):
                    continue
                j = b * S + s
                col_vocab[j] = v
                own_pairs.append((b, s, j))
                if v in placed:
                    dup_info.append((b, j, v, placed[v]))
                else:
                    placed[v] = j
        n_extra = len(col_vocab[col_vocab >= 0]) - len(placed)
        # batches with >=2 pairs of the same value need one pristine column
        batch_groups = {}
        for (b, j, v, pj) in dup_info:
            batch_groups.setdefault((b, v), 0)
        # count per (b,v) multiplicity incl. primary
        mult = {}
        for (b, s, j) in own_pairs:
            v = int(src[b, s])
            mult[(b, v)] = mult.get((b, v), 0) + 1
        pristine_needed = sorted({v for (b, v), k in mult.items() if k >= 2})
        # fill remaining vocab into free columns
        free_cols = np.nonzero(col_vocab < 0)[0]
        remaining = sorted(set(range(lo, hi)) - set(placed.keys()))
        need = len(remaining) + len(pristine_needed)
        assert need <= len(free_cols), (
            f"core {c}: need {need} cols, have {len(free_cols)}")
        pristine_col = {}
        idx = 0
        for v in remaining:
            col_vocab[free_cols[idx]] = v
            placed[v] = int(free_cols[idx])
            idx += 1
        for v in pristine_needed:
            pristine_col[v] = int(free_cols[idx])
            col_vocab[free_cols[idx]] = v
            idx += 1
        # W permuted (pad cols stay zero)
        wTp = np.zeros((H, WCOLS), np.float32)
        valid = col_vocab >= 0
        wTp[:, valid] = W_gen.astype(np.float32)[col_vocab[valid], :].T

        # ownership mask [128, TT_N*S]: row p of tile tt is token
        # n = tt*128 + p (batch b = 2*tt + p//64); col group tt, col s
        om = np.zeros((128, TT_N * S), np.float32)
        for (b, s, j) in own_pairs:
            tt, half = b // 2, b % 2
            om[64 * half:64 * half + 64, tt * S + s] = 1.0

        per_core.append(dict(
            in_map={"hT": hT, "cT": cT, "wT": np.ascontiguousarray(wTp),
                    "wcp": wcp, "bcp": bcp, "amask": amask, "omask": om},
            col_vocab=col_vocab, mult=mult, placed=placed,
            own_pairs=own_pairs, pristine_col=pristine_col,
        ))
    return per_core


def _assemble(per_core, results):
    """results[c]['out'] is [NTOK, WCOLS] (token n = b*64+t). Returns the
    full [T, B, V] float32 output."""
    big = np.empty((NTOK, V), np.float32)
    for c in range(NCORE):
        o = results[c]["out"]
        meta = per_core[c]
        col_vocab = meta["col_vocab"]
        valid = np.nonzero(col_vocab >= 0)[0]
        # base: for each vocab value use its primary column
        prim = meta["placed"]  # vocab -> col
        vids = np.fromiter(prim.keys(), np.int64, len(prim))
        cols = np.fromiter((prim[int(v)] for v in vids), np.int64, len(vids))
        big[:, vids] = o[:, cols]
        # per-batch overrides for scattered pairs
        mult = meta["mult"]
        pair_cols = {}
        for (b, s, j) in meta["own_pairs"]:
            v = int(col_vocab[j])
            pair_cols.setdefault((b, v), []).append(j)
        for (b, v), jlist in pair_cols.items():
            rows = slice(b * T, (b + 1) * T)
            if len(jlist) == 1:
                big[rows, v] = o[rows, jlist[0]]
            else:
                # several source positions hit the same vocab in one batch:
                # columns each carry one contribution; merge in prob space.
                j0 = meta["pristine_col"][v]
                acc = np.exp(o[rows, jlist[0]].astype(np.float64))
                base = np.exp(o[rows, j0].astype(np.float64))
                for j in jlist[1:]:
                    acc += np.exp(o[rows, j].astype(np.float64)) - base
                big[rows, v] = np.log(acc).astype(np.float32)
    return np.ascontiguousarray(
        big.reshape(B, T, V).transpose(1, 0, 2)).astype(np.float32)


_PROGRAM_CACHE = {}


def _get_program():
    if "nc" not in _PROGRAM_CACHE:
        _PROGRAM_CACHE["nc"] = build_program()
    return _PROGRAM_CACHE["nc"]


def kernel(hidden, context, src, W_gen, b_gen, w_copy, b_copy):
    per_core = _prep_inputs(hidden, context, src, W_gen, b_gen, w_copy,
                            b_copy)
    nc = _get_program()
    in_maps = [pc["in_map"] for pc in per_core]
    last_err = None
    for attempt in range(3):
        try:
            res = run_bass_kernel_spmd(nc, in_maps, list(range(NCORE)))
            break
        except Exception as e:  # transient device errors: retry
            last_err = e
            if "UNRECOVERABLE" in str(e) or "UNAVAILABLE" in str(e):
                time.sleep(15)
                continue
            raise
    else:
        raise last_err
    return _assemble(per_core, res.results)
